# revision 38
# baseline (speedup 1.0000x reference)
"""Trainium2 Bass kernel for nn_Attn_Pred_Model (causal geometric-decay FIR + position biases).

Math:
  out[b,t,d] = alpha * sum_{i=0}^{P-1} beta^i * x[b,t-1-i,d]
               + pos_fwd[d] + pos_bwd[bucket(t,d)]

The FIR along the sequence dim is a banded (block-bidiagonal) Toeplitz matmul:
with 128-row sequence blocks,  y[blk] = D @ x[blk] + L @ x[blk-1]
for two constant 128x128 matrices D, L built from (alpha, beta) on the host.
Both matmuls are fused into ONE DoubleRow-perf-mode fp8 matmul per chunk:
DoubleRow computes w[:,0].T @ m[:,0] + w[:,1].T @ m[:,1] in a single pass, so
the stationary pair is (L^T, D^T) and the moving pair is (x[blk-1], x[blk]) --
adjacent slices of one big SBUF tile holding all 8 sequence blocks.

Dtypes (exact rel-err vs the reference measured host-side on the fixed bench
inputs: ~1.4e-2 against the 2e-2 gate):
  x, D, L: fp8-e4m3 (TRN FP8_EXP4, max +-240 = ml_dtypes.float8_e4m3)
  PSUM matmul output: bf16 -> the bias-add drain TT is all-16-bit and
  packed, which qualifies for the DVE 2x perf mode (0.67ns/row).
  position bias, out: bf16.

Sharding: pure data parallelism -- batch dim split across the 8 NeuronCores.
The device-side layout is (S, B_loc, NB): the shard handed to each core is a
transposed *view*; the SPMD runner's input-concat materializes it (same
one-copy cost as contiguous sharding) and in exchange every DMA descriptor
is a 4-16KB contiguous run instead of 128B.
"""

import os
import sys

import numpy as np

os.environ.setdefault("MYCRO_LOCAL_CACHE", "1")
if "/opt/trn_rl_repo" not in sys.path:
    sys.path.insert(0, "/opt/trn_rl_repo")

B, S, NB = 1024, 1024, 32
NCORES = 8
B_LOC = B // NCORES  # batches per core
SB = 128             # sequence block size
NTB = S // SB        # sequence blocks
# ISA cap: matmul moving AP <= 512 elements. DoubleRow pairs count toward
# it, so a DR matmul covers 8 batches (2x256); block 0's normal-mode
# matmuls cover 16. Four matmul chunks share one PSUM tile (one
# accumulation group over disjoint column slices), drained as one 32-batch
# TT.
BC = 8               # batches per DoubleRow matmul chunk (256 columns)
BC0 = 16             # batches per normal-mode matmul chunk (block 0)
BCD = 32             # batches per PSUM tile / drain chunk
F32 = np.float32

_PROGRAM_CACHE = {}


def _install_ntff_shim():
    """Provide antenv.axon_hooks if the image lacks it, so trace=True works.

    The axon boot module ships a ctypes NTFF-profile hook but only registers
    it when ``antenv.axon_hooks`` exists; this image's antenv does not have
    that module, which makes ``run_bass_kernel_spmd(trace=True)`` crash on
    import. Inject an in-memory equivalent. No-op if tracing is never used.
    """
    try:
        import antenv.axon_hooks  # noqa: F401
        return
    except ImportError:
        pass
    try:
        import types

        import antenv
        from trn_agent_boot.trn_boot import _ntff_profile_via_ctypes

        hook = _ntff_profile_via_ctypes("/opt/axon/libaxon_pjrt.so")
        mod = types.ModuleType("antenv.axon_hooks")
        state = {"hook": hook}
        mod.get_axon_ntff_profile_hook = lambda: state["hook"]
        mod.set_axon_ntff_profile_hook = lambda h: state.__setitem__("hook", h)
        sys.modules["antenv.axon_hooks"] = mod
        antenv.axon_hooks = mod
    except Exception:
        pass


def _enable_ldw_opt():
    """Flip walrus's --enable-ldw-opt to true for our compiles.

    The matmuls here cycle through just two stationary matrices (block 0's
    D-only and the DoubleRow L/D pair), but every self-loading InstMatmult
    re-emits an LDWEIGHTS (~105-210ns each, >100 of them). The walrus
    ldw-opt pass drops redundant consecutive loads; correctness is verified
    by the caller's rel-err check."""
    # Dead end: walrus rejects this producer's BIR with
    # "InstLdweights is not compatible with LDW optimization" when the
    # pass is enabled; redundant weight loads are instead removed by
    # _dedupe_ldweights below.
    return


def _dedupe_ldweights(nc):
    """Drop InstLdweights that reload the exact weights already resident.

    The PE stationary persists across matmuls; bass emits one InstLdweights
    per InstMatmult regardless (and walrus's own ldw-opt pass rejects this
    producer's BIR). All matmuls here cycle through just two stationaries,
    so all but the first load of each run are redundant (~167ns each, >100
    of them). Loads carrying sync are kept as same-engine NoOps so
    semaphore semantics are untouched."""
    import concourse.mybir as mybir

    for fn in nc.m.functions:
        for blk in fn.blocks:
            out = []
            last_key = None
            changed = False
            for inst in blk.instructions:
                if isinstance(inst, mybir.InstLdweights):
                    a = inst.ins[0]
                    key = (a.memref, a.offset, str(a.ap), str(a.dtype),
                           str(inst.perf_mode), inst.is_transpose)
                    if key == last_key:
                        changed = True
                        si = inst.sync_info
                        if si is not None and (si.on_wait or si.on_update):
                            out.append(mybir.InstNoOp(
                                name=f"{inst.name}-ldwskip",
                                engine=inst.engine,
                                bass_nofuse=True,
                                sync_info=si,
                            ))
                        continue
                    last_key = key
                elif isinstance(inst, mybir.InstMatmult):
                    pass  # matmults don't disturb the stationary
                elif inst.engine == getattr(mybir.EngineType, "PE",
                                            inst.engine):
                    pass
                out.append(inst)
            if changed:
                blk.instructions = out
    return nc


def _split_multi_waits(nc, maxw=1):
    """Work around a walrus limit in this image: instructions carrying more
    than ~2 sem waits die in codegen with "Too many sync wait commands".
    Move excess waits onto same-engine NoOps placed just before the
    instruction (identical sync semantics, negligible cost)."""
    import concourse.mybir as mybir

    for fn in nc.m.functions:
        for blk in fn.blocks:
            out = []
            changed = False
            for inst in blk.instructions:
                si = inst.sync_info
                if si is not None and len(si.on_wait) > maxw:
                    waits = list(si.on_wait)
                    excess, keep = waits[:-maxw], waits[-maxw:]
                    for k, w in enumerate(excess):
                        out.append(mybir.InstNoOp(
                            name=f"{inst.name}-sw{k}",
                            engine=inst.engine,
                            bass_nofuse=True,
                            sync_info=mybir.SyncInfo(on_wait=[w], on_update=[]),
                        ))
                    inst.sync_info = mybir.SyncInfo(
                        on_wait=list(keep), on_update=list(si.on_update))
                    changed = True
                out.append(inst)
            if changed:
                blk.instructions = out
    return nc


def build_program(b_loc=B_LOC, split_waits=True):
    """Per-core Bass/Tile program. Device-side x/out layout is (S, b_loc, NB).

    Measured HW facts this schedule is built around:
      - the two hardware-DGE rings (SP+ACT) sustain ~420-438 GB/s
        combined when both stream; input blocks alternate between them
        and output halves ride behind, paced with a one-block lookahead
        (pre-issuing everything measurably slows the rings);
      - consts ride the otherwise-idle gpsimd SWDGE ring so neither
        HWDGE ring has const descriptors ahead of the x stream;
      - the PE runs at 1.2GHz until it has been continuously busy for
        ~3us (then 2.4GHz), so bursts must be long and uninterrupted:
        one DoubleRow matmul per 2048-column chunk, one LDWEIGHTS each;
      - the bf16-PSUM drain TT on DVE qualifies for 2x mode, so DVE
        alone drains a block (2x 1365ns) inside the ~3us block period.

    split_waits=True post-processes for the HW compiler; pass False when the
    module is destined for CoreSim (the sim rejects the injected NoOps)."""
    import concourse.bass as bass
    import concourse.mybir as mybir
    import concourse.tile as tile

    f32 = mybir.dt.float32
    bf16 = mybir.dt.bfloat16
    fp8 = mybir.dt.float8e4
    ndrain = b_loc // BCD         # drain chunks (PSUM tiles) per block
    nmm = BCD // BC               # DoubleRow matmuls per PSUM tile
    nmm0 = BCD // BC0             # normal matmuls per PSUM tile (block 0)

    nc = bass.Bass("TRN2")
    x_h = nc.declare_dram_parameter("x", [S, b_loc * NB], fp8, False)
    # stationary pair for DoubleRow: slot 0 = L^T, slot 1 = D^T
    dl_h = nc.declare_dram_parameter("dlmat", [SB, 2, SB], fp8, False)
    # pbias pre-transposed on host -> contiguous 512B-per-partition DMA
    pb_h = nc.declare_dram_parameter("pbias", [SB, NTB, NB], bf16, False)
    out_h = nc.declare_dram_parameter("out", [S, b_loc, NB], bf16, True)

    with tile.TileContext(nc) as tc:
        with (
            tc.tile_pool(name="consts", bufs=1) as cpool,
            tc.tile_pool(name="outp", bufs=6) as opool,
            tc.tile_pool(name="tmp", bufs=6) as tpool,
            tc.tile_pool(name="psum", bufs=4, space="PSUM") as ppool,
        ):
            dl_sb = cpool.tile([SB, 2, SB], fp8, tag="dl")
            pb_sb = cpool.tile([SB, NTB, NB], bf16, tag="pb")
            # consts at the head of the then-idle scalar ring: tiny
            # (~24KB) so they cost ~0.1us of ring time
            nc.scalar.dma_start(dl_sb[:], dl_h[:])
            nc.scalar.dma_start(pb_sb[:], pb_h[:])

            # one resident tile holds all NTB sequence blocks so the
            # DoubleRow moving pair (x[blk-1], x[blk]) is a contiguous
            # [SB, 2, CW] slice
            xall = cpool.tile([SB, NTB, b_loc * NB], fp8, tag="xall")

            hb = b_loc // 2  # half-block batch split for finer DMA/sync
            DR = mybir.MatmulPerfMode.DoubleRow

            def issue_in(tb):
                # even blocks on the sync queue, odd on scalar; paced
                # one block per iteration so neither ring backs up into
                # its engine.
                eng = nc.sync if tb % 2 == 0 else nc.scalar
                r_ = slice(tb * SB, (tb + 1) * SB)
                if tb in (0, 1):
                    # first two blocks ride BOTH rings (half each) so
                    # compute starts as soon as possible
                    o_ = nc.scalar if tb % 2 == 0 else nc.sync
                    eng.dma_start(xall[:, tb, :hb * NB], x_h[r_, :hb * NB])
                    o_.dma_start(xall[:, tb, hb * NB:], x_h[r_, hb * NB:])
                elif tb == NTB - 1:
                    # split the last block so its drain starts before
                    # the full block lands
                    eng.dma_start(xall[:, tb, :hb * NB], x_h[r_, :hb * NB])
                    eng.dma_start(xall[:, tb, hb * NB:], x_h[r_, hb * NB:])
                else:
                    eng.dma_start(xall[:, tb, :], x_h[r_])

            issue_in(0)
            issue_in(1)
            for tb in range(NTB):
                for nx in (2 * tb + 2, 2 * tb + 3):
                    if nx < NTB:
                        issue_in(nx)
                r = slice(tb * SB, (tb + 1) * SB)
                ot = opool.tile([SB, b_loc, NB], bf16, tag="ot")
                bias = pb_sb[:, tb:tb + 1, :].broadcast_to((SB, BCD, NB))
                pss = {}
                for c in range(ndrain):
                    ps = ppool.tile([SB, BCD, NB], f32, tag="ps")
                    # matmuls write disjoint column slices of the tile;
                    # start=True zeroes only the 2KB PSUM BANK containing
                    # the slice, so the first matmul touching each bank
                    # must carry start (bank = 16 batches of f32)
                    if tb == 0:
                        for m in range(nmm0):
                            cs = slice((c * BCD + m * BC0) * NB,
                                       (c * BCD + (m + 1) * BC0) * NB)
                            nc.tensor.matmul(
                                ps[:, m * BC0:(m + 1) * BC0, :],
                                dl_sb[:, 1:2, :], xall[:, 0:1, cs],
                                start=True, stop=True,
                                skip_group_check=True)
                    else:
                        for m in range(nmm):
                            cs = slice((c * BCD + m * BC) * NB,
                                       (c * BCD + (m + 1) * BC) * NB)
                            nc.tensor.matmul(
                                ps[:, m * BC:(m + 1) * BC, :],
                                dl_sb[:], xall[:, tb - 1:tb + 1, cs],
                                start=(m % 2 == 0), stop=(m % 2 == 1),
                                skip_group_check=True, perf_mode=DR)
                    pss[c] = ps
                # PSUM -> SBUF bias-add (measured: DVE TT from PSUM f32
                # 1215ns/chunk, GP TT 2123ns, ACT stage-copy 1113ns).
                # DVE saturates first (it paces the whole conveyor and
                # pushes the output flush into an 11us tail), so GP
                # takes c0 every block plus c2 on even blocks -- c0/c2
                # sit in different output halves, spreading the chain
                # latency instead of doubling one half's gate. GPSIMD
                # cannot read PSUM so its chunks stage via ACT.
                gp_chunks = (0, 2) if tb % 2 == 0 else (0,)
                for c in range(ndrain):
                    bs = slice(c * BCD, (c + 1) * BCD)
                    if c in gp_chunks:
                        tmp = tpool.tile([SB, BCD, NB], bf16, tag="tmp")
                        nc.scalar.copy(tmp[:], pss[c][:])
                        nc.gpsimd.tensor_tensor(ot[:, bs, :], tmp[:], bias,
                                                mybir.AluOpType.add)
                    else:
                        nc.vector.tensor_tensor(ot[:, bs, :], pss[c][:], bias,
                                                mybir.AluOpType.add)
                # output halves: h0 rides scalar, h1 rides sync -- each
                # behind that queue's remaining input blocks, which are
                # all wait-free and drain first. The last block's
                # outputs go out in quarters so the ring starts as soon
                # as the first drain lands.
                if tb == NTB - 1:
                    for q in range(4):
                        qs = slice(q * BCD, (q + 1) * BCD)
                        oeng = nc.scalar if q < 2 else nc.sync
                        oeng.dma_start(out_h[r, qs, :], ot[:, qs, :])
                else:
                    nc.scalar.dma_start(out_h[r, :hb, :], ot[:, :hb, :])
                    nc.sync.dma_start(out_h[r, hb:, :], ot[:, hb:, :])
    nc = _dedupe_ldweights(nc)
    return _split_multi_waits(nc) if split_waits else nc


def to_bf16(a):
    """Convert to bfloat16 (ml_dtypes)."""
    import ml_dtypes

    return np.ascontiguousarray(np.asarray(a, dtype=F32)).astype(
        ml_dtypes.bfloat16)


def to_fp8(a):
    """Convert to TRN fp8-e4m3 (bias 7, max +-240 = ml_dtypes.float8_e4m3).

    Clip to +-240 first: values past the TRN max would round to inf."""
    import ml_dtypes

    a = np.clip(np.asarray(a, dtype=F32), -240.0, 240.0)
    return np.ascontiguousarray(a).astype(ml_dtypes.float8_e4m3)


def host_consts(alpha, beta, pos_fwd_param, pos_bwd_param, past_steps):
    """Precompute the (L^T, D^T) stationary pair and the position bias."""
    P = int(np.asarray(past_steps).reshape(-1)[0]) if np.ndim(past_steps) else int(past_steps)
    assert P <= SB, f"past_steps {P} > block size {SB} unsupported"
    a = float(np.asarray(alpha).reshape(-1)[0])
    b = float(np.asarray(beta).reshape(-1)[0])
    w = a * np.power(b, np.arange(P, dtype=np.float64))

    idx = np.arange(SB)
    km = idx[:, None] - idx[None, :]          # t - s
    D = np.where((km >= 1) & (km <= P), w[np.clip(km - 1, 0, P - 1)], 0.0)
    kml = km + SB                             # cross-block: t - s + 128
    L = np.where((kml >= 1) & (kml <= P), w[np.clip(kml - 1, 0, P - 1)], 0.0)
    # DoubleRow stationary layout [K, 2, M]: slot 0 pairs with the
    # moving slot 0 (x[blk-1]) -> L^T; slot 1 with x[blk] -> D^T
    dlpack = to_fp8(np.stack([L.T, D.T], axis=1))  # (SB, 2, SB)

    t = np.arange(S)[:, None]
    j = np.arange(NB)[None, :]
    bucket = ((t - NB * j) % S) // NB         # (S, NB)
    pf = np.asarray(pos_fwd_param, dtype=np.float64).reshape(NB)
    pbw = np.asarray(pos_bwd_param, dtype=np.float64).reshape(NB)
    pb = pf[None, :] + pbw[bucket]            # (S, NB)
    # pbias pre-transposed to (SB, NTB, NB) on the host
    pbias = to_bf16(pb.reshape(NTB, SB, NB).transpose(1, 0, 2))
    return dlpack, pbias


def reference_numpy(x, alpha, beta, pos_fwd_param, pos_bwd_param, past_steps):
    """Float64 host reference (for self-tests)."""
    P = int(past_steps)
    a = float(np.asarray(alpha).reshape(-1)[0])
    b = float(np.asarray(beta).reshape(-1)[0])
    w = a * np.power(b, np.arange(P, dtype=np.float64))
    xf = np.asarray(x, dtype=np.float64)
    Bn, Sn, Dn = xf.shape
    y = np.zeros_like(xf)
    for i in range(P):
        y[:, i + 1:, :] += w[i] * xf[:, :Sn - 1 - i, :]
    t = np.arange(Sn)[:, None]
    j = np.arange(Dn)[None, :]
    bucket = ((t - Dn * j) % Sn) // Dn
    pf = np.asarray(pos_fwd_param, dtype=np.float64).reshape(Dn)
    pbw = np.asarray(pos_bwd_param, dtype=np.float64).reshape(Dn)
    return y + pf[None, :] + pbw[bucket]


def kernel(x, alpha, beta, pos_fwd_param, pos_bwd_param, past_steps):
    _install_ntff_shim()
    _enable_ldw_opt()
    from concourse.bass_utils import run_bass_kernel_spmd

    x = np.asarray(x)
    assert x.shape == (B, S, NB), x.shape
    x = to_fp8(x)  # device datapath is fp8; halves HBM read traffic vs bf16
    dlpack, pbias = host_consts(alpha, beta, pos_fwd_param,
                                pos_bwd_param, past_steps)

    if "hw" not in _PROGRAM_CACHE:
        _PROGRAM_CACHE["hw"] = build_program(B_LOC)
    nc = _PROGRAM_CACHE["hw"]

    core_ids = list(range(NCORES))
    in_maps = [
        {
            # transposed view (S, B_LOC*NB); materialized by the runner's
            # input concat -- no extra host copy vs contiguous sharding
            "x": x[i * B_LOC:(i + 1) * B_LOC].transpose(1, 0, 2).reshape(
                S, B_LOC * NB),
            "dlmat": dlpack,
            "pbias": pbias,
        }
        for i in core_ids
    ]
    res = run_bass_kernel_spmd(nc, in_maps, core_ids)
    out = np.empty((B, S, NB), dtype=F32)
    for i in core_ids:
        out[i * B_LOC:(i + 1) * B_LOC] = (
            res.results[i]["out"].astype(F32).transpose(1, 0, 2))
    if res.exec_time_ns is not None:
        kernel.last_exec_time_ns = res.exec_time_ns
    kernel.last_results = res
    return out


kernel.last_exec_time_ns = None
kernel.last_results = None


# revision 39
# speedup vs baseline: 1.1007x; 1.1007x over previous
"""Trainium2 Bass kernel for nn_Attn_Pred_Model (causal geometric-decay FIR + position biases).

Math:
  out[b,t,d] = alpha * sum_{i=0}^{P-1} beta^i * x[b,t-1-i,d]
               + pos_fwd[d] + pos_bwd[bucket(t,d)]

The FIR along the sequence dim is a banded (block-bidiagonal) Toeplitz matmul:
with 128-row sequence blocks,  y[blk] = D @ x[blk] + L @ x[blk-1]
for two constant 128x128 matrices D, L built from (alpha, beta) on the host.
Both matmuls are fused into ONE DoubleRow-perf-mode fp8 matmul per chunk:
DoubleRow computes w[:,0].T @ m[:,0] + w[:,1].T @ m[:,1] in a single pass, so
the stationary pair is (L^T, D^T) and the moving pair is (x[blk-1], x[blk]) --
adjacent slices of one big SBUF tile holding all 8 sequence blocks.

Dtypes (exact rel-err vs the reference measured host-side on the fixed bench
inputs: ~1.4e-2 against the 2e-2 gate):
  x, D, L: fp8-e4m3 (TRN FP8_EXP4, max +-240 = ml_dtypes.float8_e4m3)
  PSUM matmul output: bf16 -> the bias-add drain TT is all-16-bit and
  packed, which qualifies for the DVE 2x perf mode (0.67ns/row).
  position bias, out: bf16.

Sharding: pure data parallelism -- batch dim split across the 8 NeuronCores.
The device-side layout is (S, B_loc, NB): the shard handed to each core is a
transposed *view*; the SPMD runner's input-concat materializes it (same
one-copy cost as contiguous sharding) and in exchange every DMA descriptor
is a 4-16KB contiguous run instead of 128B.
"""

import os
import sys

import numpy as np

os.environ.setdefault("MYCRO_LOCAL_CACHE", "1")
if "/opt/trn_rl_repo" not in sys.path:
    sys.path.insert(0, "/opt/trn_rl_repo")

B, S, NB = 1024, 1024, 32
NCORES = 8
B_LOC = B // NCORES  # batches per core
SB = 128             # sequence block size
NTB = S // SB        # sequence blocks
# ISA cap: matmul moving AP <= 512 elements. DoubleRow pairs count toward
# it, so a DR matmul covers 8 batches (2x256); block 0's normal-mode
# matmuls cover 16. Four matmul chunks share one PSUM tile (one
# accumulation group over disjoint column slices), drained as one 32-batch
# TT.
BC = 8               # batches per DoubleRow matmul chunk (256 columns)
BC0 = 16             # batches per normal-mode matmul chunk (block 0)
BCD = 32             # batches per PSUM tile / drain chunk
F32 = np.float32

_PROGRAM_CACHE = {}


def _install_ntff_shim():
    """Provide antenv.axon_hooks if the image lacks it, so trace=True works.

    The axon boot module ships a ctypes NTFF-profile hook but only registers
    it when ``antenv.axon_hooks`` exists; this image's antenv does not have
    that module, which makes ``run_bass_kernel_spmd(trace=True)`` crash on
    import. Inject an in-memory equivalent. No-op if tracing is never used.
    """
    try:
        import antenv.axon_hooks  # noqa: F401
        return
    except ImportError:
        pass
    try:
        import types

        import antenv
        from trn_agent_boot.trn_boot import _ntff_profile_via_ctypes

        hook = _ntff_profile_via_ctypes("/opt/axon/libaxon_pjrt.so")
        mod = types.ModuleType("antenv.axon_hooks")
        state = {"hook": hook}
        mod.get_axon_ntff_profile_hook = lambda: state["hook"]
        mod.set_axon_ntff_profile_hook = lambda h: state.__setitem__("hook", h)
        sys.modules["antenv.axon_hooks"] = mod
        antenv.axon_hooks = mod
    except Exception:
        pass


def _enable_ldw_opt():
    """Flip walrus's --enable-ldw-opt to true for our compiles.

    The matmuls here cycle through just two stationary matrices (block 0's
    D-only and the DoubleRow L/D pair), but every self-loading InstMatmult
    re-emits an LDWEIGHTS (~105-210ns each, >100 of them). The walrus
    ldw-opt pass drops redundant consecutive loads; correctness is verified
    by the caller's rel-err check."""
    # Dead end: walrus rejects this producer's BIR with
    # "InstLdweights is not compatible with LDW optimization" when the
    # pass is enabled; redundant weight loads are instead removed by
    # _dedupe_ldweights below.
    return


def _dedupe_ldweights(nc):
    """Drop InstLdweights that reload the exact weights already resident.

    The PE stationary persists across matmuls; bass emits one InstLdweights
    per InstMatmult regardless (and walrus's own ldw-opt pass rejects this
    producer's BIR). All matmuls here cycle through just two stationaries,
    so all but the first load of each run are redundant (~167ns each, >100
    of them). Loads carrying sync are kept as same-engine NoOps so
    semaphore semantics are untouched."""
    import concourse.mybir as mybir

    for fn in nc.m.functions:
        for blk in fn.blocks:
            out = []
            last_key = None
            changed = False
            for inst in blk.instructions:
                if isinstance(inst, mybir.InstLdweights):
                    a = inst.ins[0]
                    key = (a.memref, a.offset, str(a.ap), str(a.dtype),
                           str(inst.perf_mode), inst.is_transpose)
                    if key == last_key:
                        changed = True
                        si = inst.sync_info
                        if si is not None and (si.on_wait or si.on_update):
                            out.append(mybir.InstNoOp(
                                name=f"{inst.name}-ldwskip",
                                engine=inst.engine,
                                bass_nofuse=True,
                                sync_info=si,
                            ))
                        continue
                    last_key = key
                elif isinstance(inst, mybir.InstMatmult):
                    pass  # matmults don't disturb the stationary
                elif inst.engine == getattr(mybir.EngineType, "PE",
                                            inst.engine):
                    pass
                out.append(inst)
            if changed:
                blk.instructions = out
    return nc


def _split_multi_waits(nc, maxw=1):
    """Work around a walrus limit in this image: instructions carrying more
    than ~2 sem waits die in codegen with "Too many sync wait commands".
    Move excess waits onto same-engine NoOps placed just before the
    instruction (identical sync semantics, negligible cost)."""
    import concourse.mybir as mybir

    for fn in nc.m.functions:
        for blk in fn.blocks:
            out = []
            changed = False
            for inst in blk.instructions:
                si = inst.sync_info
                if si is not None and len(si.on_wait) > maxw:
                    waits = list(si.on_wait)
                    excess, keep = waits[:-maxw], waits[-maxw:]
                    for k, w in enumerate(excess):
                        out.append(mybir.InstNoOp(
                            name=f"{inst.name}-sw{k}",
                            engine=inst.engine,
                            bass_nofuse=True,
                            sync_info=mybir.SyncInfo(on_wait=[w], on_update=[]),
                        ))
                    inst.sync_info = mybir.SyncInfo(
                        on_wait=list(keep), on_update=list(si.on_update))
                    changed = True
                out.append(inst)
            if changed:
                blk.instructions = out
    return nc


def build_program(b_loc=B_LOC, split_waits=True):
    """Per-core Bass/Tile program. Device-side x/out layout is (S, b_loc, NB).

    Measured HW facts this schedule is built around:
      - the two hardware-DGE rings (SP+ACT) sustain ~420-438 GB/s
        combined when both stream; input blocks alternate between them
        and output halves ride behind, paced with a one-block lookahead
        (pre-issuing everything measurably slows the rings);
      - consts ride the otherwise-idle gpsimd SWDGE ring so neither
        HWDGE ring has const descriptors ahead of the x stream;
      - the PE runs at 1.2GHz until it has been continuously busy for
        ~3us (then 2.4GHz), so bursts must be long and uninterrupted:
        one DoubleRow matmul per 2048-column chunk, one LDWEIGHTS each;
      - the bf16-PSUM drain TT on DVE qualifies for 2x mode, so DVE
        alone drains a block (2x 1365ns) inside the ~3us block period.

    split_waits=True post-processes for the HW compiler; pass False when the
    module is destined for CoreSim (the sim rejects the injected NoOps)."""
    import concourse.bass as bass
    import concourse.mybir as mybir
    import concourse.tile as tile

    f32 = mybir.dt.float32
    bf16 = mybir.dt.bfloat16
    fp8 = mybir.dt.float8e4
    ndrain = b_loc // BCD         # drain chunks (PSUM tiles) per block
    nmm = BCD // BC               # DoubleRow matmuls per PSUM tile
    nmm0 = BCD // BC0             # normal matmuls per PSUM tile (block 0)

    nc = bass.Bass("TRN2")
    x_h = nc.declare_dram_parameter("x", [S, b_loc * NB], fp8, False)
    # stationary pair for DoubleRow: slot 0 = L^T, slot 1 = D^T
    dl_h = nc.declare_dram_parameter("dlmat", [SB, 2, SB], fp8, False)
    # pbias pre-transposed on host -> contiguous 512B-per-partition DMA
    pb_h = nc.declare_dram_parameter("pbias", [SB, NTB, NB], bf16, False)
    out_h = nc.declare_dram_parameter("out", [S, b_loc, NB], bf16, True)

    with tile.TileContext(nc) as tc:
        with (
            tc.tile_pool(name="consts", bufs=1) as cpool,
            tc.tile_pool(name="outp", bufs=6) as opool,
            tc.tile_pool(name="tmp", bufs=6) as tpool,
            tc.tile_pool(name="psum", bufs=4, space="PSUM") as ppool,
        ):
            dl_sb = cpool.tile([SB, 2, SB], fp8, tag="dl")
            pb_sb = cpool.tile([SB, NTB, NB], bf16, tag="pb")
            # consts at the head of the then-idle scalar ring: tiny
            # (~24KB) so they cost ~0.1us of ring time
            nc.scalar.dma_start(dl_sb[:], dl_h[:])
            nc.scalar.dma_start(pb_sb[:], pb_h[:])

            # one resident tile holds all NTB sequence blocks so the
            # DoubleRow moving pair (x[blk-1], x[blk]) is a contiguous
            # [SB, 2, CW] slice
            xall = cpool.tile([SB, NTB, b_loc * NB], fp8, tag="xall")

            hb = b_loc // 2  # half-block batch split for finer DMA/sync
            DR = mybir.MatmulPerfMode.DoubleRow

            def issue_in(tb):
                # even blocks on the sync queue, odd on scalar; paced
                # one block per iteration so neither ring backs up into
                # its engine.
                eng = nc.sync if tb % 2 == 0 else nc.scalar
                r_ = slice(tb * SB, (tb + 1) * SB)
                if tb in (0, 1):
                    # first two blocks ride BOTH rings (half each) so
                    # compute starts as soon as possible
                    o_ = nc.scalar if tb % 2 == 0 else nc.sync
                    eng.dma_start(xall[:, tb, :hb * NB], x_h[r_, :hb * NB])
                    o_.dma_start(xall[:, tb, hb * NB:], x_h[r_, hb * NB:])
                elif tb == NTB - 1:
                    # split the last block so its drain starts before
                    # the full block lands
                    eng.dma_start(xall[:, tb, :hb * NB], x_h[r_, :hb * NB])
                    eng.dma_start(xall[:, tb, hb * NB:], x_h[r_, hb * NB:])
                else:
                    eng.dma_start(xall[:, tb, :], x_h[r_])

            issue_in(0)
            issue_in(1)
            for tb in range(NTB):
                for nx in (2 * tb + 2, 2 * tb + 3):
                    if nx < NTB:
                        issue_in(nx)
                r = slice(tb * SB, (tb + 1) * SB)
                ot = opool.tile([SB, b_loc, NB], bf16, tag="ot")
                bias = pb_sb[:, tb:tb + 1, :].broadcast_to((SB, BCD, NB))
                pss = {}
                for c in range(ndrain):
                    ps = ppool.tile([SB, BCD, NB], f32, tag="ps")
                    # matmuls write disjoint column slices of the tile;
                    # start=True zeroes only the 2KB PSUM BANK containing
                    # the slice, so the first matmul touching each bank
                    # must carry start (bank = 16 batches of f32)
                    if tb == 0:
                        for m in range(nmm0):
                            cs = slice((c * BCD + m * BC0) * NB,
                                       (c * BCD + (m + 1) * BC0) * NB)
                            nc.tensor.matmul(
                                ps[:, m * BC0:(m + 1) * BC0, :],
                                dl_sb[:, 1:2, :], xall[:, 0:1, cs],
                                start=True, stop=True,
                                skip_group_check=True)
                    else:
                        for m in range(nmm):
                            cs = slice((c * BCD + m * BC) * NB,
                                       (c * BCD + (m + 1) * BC) * NB)
                            nc.tensor.matmul(
                                ps[:, m * BC:(m + 1) * BC, :],
                                dl_sb[:], xall[:, tb - 1:tb + 1, cs],
                                start=(m % 2 == 0), stop=(m % 2 == 1),
                                skip_group_check=True, perf_mode=DR)
                    pss[c] = ps
                # PSUM -> SBUF bias-add (measured: DVE TT from PSUM f32
                # 1215ns/chunk, GP TT 2123ns, ACT stage-copy 1113ns).
                # GPSIMD cannot read PSUM so its chunk stages via ACT;
                # it takes c0 only (the slow path first). Giving GP a
                # second chunk per block measures consistently WORSE
                # (~54us vs ~49us): the ACT->GP chain latency outweighs
                # the DVE relief.
                gp_chunks = (0,)
                for c in range(ndrain):
                    bs = slice(c * BCD, (c + 1) * BCD)
                    if c in gp_chunks:
                        tmp = tpool.tile([SB, BCD, NB], bf16, tag="tmp")
                        nc.scalar.copy(tmp[:], pss[c][:])
                        nc.gpsimd.tensor_tensor(ot[:, bs, :], tmp[:], bias,
                                                mybir.AluOpType.add)
                    else:
                        nc.vector.tensor_tensor(ot[:, bs, :], pss[c][:], bias,
                                                mybir.AluOpType.add)
                # output halves: h0 rides scalar, h1 rides sync -- each
                # behind that queue's remaining input blocks, which are
                # all wait-free and drain first. The last block's
                # outputs go out in quarters so the ring starts as soon
                # as the first drain lands.
                if tb == NTB - 1:
                    for q in range(4):
                        qs = slice(q * BCD, (q + 1) * BCD)
                        oeng = nc.scalar if q < 2 else nc.sync
                        oeng.dma_start(out_h[r, qs, :], ot[:, qs, :])
                else:
                    nc.scalar.dma_start(out_h[r, :hb, :], ot[:, :hb, :])
                    nc.sync.dma_start(out_h[r, hb:, :], ot[:, hb:, :])
    nc = _dedupe_ldweights(nc)
    return _split_multi_waits(nc) if split_waits else nc


def to_bf16(a):
    """Convert to bfloat16 (ml_dtypes)."""
    import ml_dtypes

    return np.ascontiguousarray(np.asarray(a, dtype=F32)).astype(
        ml_dtypes.bfloat16)


def to_fp8(a):
    """Convert to TRN fp8-e4m3 (bias 7, max +-240 = ml_dtypes.float8_e4m3).

    Clip to +-240 first: values past the TRN max would round to inf."""
    import ml_dtypes

    a = np.clip(np.asarray(a, dtype=F32), -240.0, 240.0)
    return np.ascontiguousarray(a).astype(ml_dtypes.float8_e4m3)


def host_consts(alpha, beta, pos_fwd_param, pos_bwd_param, past_steps):
    """Precompute the (L^T, D^T) stationary pair and the position bias."""
    P = int(np.asarray(past_steps).reshape(-1)[0]) if np.ndim(past_steps) else int(past_steps)
    assert P <= SB, f"past_steps {P} > block size {SB} unsupported"
    a = float(np.asarray(alpha).reshape(-1)[0])
    b = float(np.asarray(beta).reshape(-1)[0])
    w = a * np.power(b, np.arange(P, dtype=np.float64))

    idx = np.arange(SB)
    km = idx[:, None] - idx[None, :]          # t - s
    D = np.where((km >= 1) & (km <= P), w[np.clip(km - 1, 0, P - 1)], 0.0)
    kml = km + SB                             # cross-block: t - s + 128
    L = np.where((kml >= 1) & (kml <= P), w[np.clip(kml - 1, 0, P - 1)], 0.0)
    # DoubleRow stationary layout [K, 2, M]: slot 0 pairs with the
    # moving slot 0 (x[blk-1]) -> L^T; slot 1 with x[blk] -> D^T
    dlpack = to_fp8(np.stack([L.T, D.T], axis=1))  # (SB, 2, SB)

    t = np.arange(S)[:, None]
    j = np.arange(NB)[None, :]
    bucket = ((t - NB * j) % S) // NB         # (S, NB)
    pf = np.asarray(pos_fwd_param, dtype=np.float64).reshape(NB)
    pbw = np.asarray(pos_bwd_param, dtype=np.float64).reshape(NB)
    pb = pf[None, :] + pbw[bucket]            # (S, NB)
    # pbias pre-transposed to (SB, NTB, NB) on the host
    pbias = to_bf16(pb.reshape(NTB, SB, NB).transpose(1, 0, 2))
    return dlpack, pbias


def reference_numpy(x, alpha, beta, pos_fwd_param, pos_bwd_param, past_steps):
    """Float64 host reference (for self-tests)."""
    P = int(past_steps)
    a = float(np.asarray(alpha).reshape(-1)[0])
    b = float(np.asarray(beta).reshape(-1)[0])
    w = a * np.power(b, np.arange(P, dtype=np.float64))
    xf = np.asarray(x, dtype=np.float64)
    Bn, Sn, Dn = xf.shape
    y = np.zeros_like(xf)
    for i in range(P):
        y[:, i + 1:, :] += w[i] * xf[:, :Sn - 1 - i, :]
    t = np.arange(Sn)[:, None]
    j = np.arange(Dn)[None, :]
    bucket = ((t - Dn * j) % Sn) // Dn
    pf = np.asarray(pos_fwd_param, dtype=np.float64).reshape(Dn)
    pbw = np.asarray(pos_bwd_param, dtype=np.float64).reshape(Dn)
    return y + pf[None, :] + pbw[bucket]


def kernel(x, alpha, beta, pos_fwd_param, pos_bwd_param, past_steps):
    _install_ntff_shim()
    _enable_ldw_opt()
    from concourse.bass_utils import run_bass_kernel_spmd

    x = np.asarray(x)
    assert x.shape == (B, S, NB), x.shape
    x = to_fp8(x)  # device datapath is fp8; halves HBM read traffic vs bf16
    dlpack, pbias = host_consts(alpha, beta, pos_fwd_param,
                                pos_bwd_param, past_steps)

    if "hw" not in _PROGRAM_CACHE:
        _PROGRAM_CACHE["hw"] = build_program(B_LOC)
    nc = _PROGRAM_CACHE["hw"]

    core_ids = list(range(NCORES))
    in_maps = [
        {
            # transposed view (S, B_LOC*NB); materialized by the runner's
            # input concat -- no extra host copy vs contiguous sharding
            "x": x[i * B_LOC:(i + 1) * B_LOC].transpose(1, 0, 2).reshape(
                S, B_LOC * NB),
            "dlmat": dlpack,
            "pbias": pbias,
        }
        for i in core_ids
    ]
    res = run_bass_kernel_spmd(nc, in_maps, core_ids)
    out = np.empty((B, S, NB), dtype=F32)
    for i in core_ids:
        out[i * B_LOC:(i + 1) * B_LOC] = (
            res.results[i]["out"].astype(F32).transpose(1, 0, 2))
    if res.exec_time_ns is not None:
        kernel.last_exec_time_ns = res.exec_time_ns
    kernel.last_results = res
    return out


kernel.last_exec_time_ns = None
kernel.last_results = None


# revision 40
# speedup vs baseline: 1.1128x; 1.0110x over previous
"""Trainium2 Bass kernel for nn_Attn_Pred_Model (causal geometric-decay FIR + position biases).

Math:
  out[b,t,d] = alpha * sum_{i=0}^{P-1} beta^i * x[b,t-1-i,d]
               + pos_fwd[d] + pos_bwd[bucket(t,d)]

The FIR along the sequence dim is a banded (block-bidiagonal) Toeplitz matmul:
with 128-row sequence blocks,  y[blk] = D @ x[blk] + L @ x[blk-1]
for two constant 128x128 matrices D, L built from (alpha, beta) on the host.
Both matmuls are fused into ONE DoubleRow-perf-mode fp8 matmul per chunk:
DoubleRow computes w[:,0].T @ m[:,0] + w[:,1].T @ m[:,1] in a single pass, so
the stationary pair is (L^T, D^T) and the moving pair is (x[blk-1], x[blk]) --
adjacent slices of one big SBUF tile holding all 8 sequence blocks.

Dtypes (exact rel-err vs the reference measured on the fixed bench
inputs: 1.19e-2 against the 2e-2 gate):
  x, D, L: fp8-e4m3 (TRN FP8_EXP4, max +-240 = ml_dtypes.float8_e4m3)
  PSUM matmul accumulate: f32 (the API requires f32 matmul output);
  position bias, out: bf16.

Sharding: pure data parallelism -- batch dim split across the 8 NeuronCores.
The device-side layout is (S, B_loc, NB): the shard handed to each core is a
transposed *view*; the SPMD runner's input-concat materializes it (same
one-copy cost as contiguous sharding) and in exchange every DMA descriptor
is a 4-16KB contiguous run instead of 128B.
"""

import os
import sys

import numpy as np

os.environ.setdefault("MYCRO_LOCAL_CACHE", "1")
if "/opt/trn_rl_repo" not in sys.path:
    sys.path.insert(0, "/opt/trn_rl_repo")

B, S, NB = 1024, 1024, 32
NCORES = 8
B_LOC = B // NCORES  # batches per core
SB = 128             # sequence block size
NTB = S // SB        # sequence blocks
# ISA cap: matmul moving AP <= 512 elements. DoubleRow pairs count toward
# it, so a DR matmul covers 8 batches (2x256); block 0's normal-mode
# matmuls cover 16. Four matmul chunks share one PSUM tile (one
# accumulation group over disjoint column slices), drained as one 32-batch
# TT.
BC = 8               # batches per DoubleRow matmul chunk (256 columns)
BC0 = 16             # batches per normal-mode matmul chunk (block 0)
BCD = 32             # batches per PSUM tile / drain chunk
F32 = np.float32

_PROGRAM_CACHE = {}


def _install_ntff_shim():
    """Provide antenv.axon_hooks if the image lacks it, so trace=True works.

    The axon boot module ships a ctypes NTFF-profile hook but only registers
    it when ``antenv.axon_hooks`` exists; this image's antenv does not have
    that module, which makes ``run_bass_kernel_spmd(trace=True)`` crash on
    import. Inject an in-memory equivalent. No-op if tracing is never used.
    """
    try:
        import antenv.axon_hooks  # noqa: F401
        return
    except ImportError:
        pass
    try:
        import types

        import antenv
        from trn_agent_boot.trn_boot import _ntff_profile_via_ctypes

        hook = _ntff_profile_via_ctypes("/opt/axon/libaxon_pjrt.so")
        mod = types.ModuleType("antenv.axon_hooks")
        state = {"hook": hook}
        mod.get_axon_ntff_profile_hook = lambda: state["hook"]
        mod.set_axon_ntff_profile_hook = lambda h: state.__setitem__("hook", h)
        sys.modules["antenv.axon_hooks"] = mod
        antenv.axon_hooks = mod
    except Exception:
        pass


def _enable_ldw_opt():
    """Flip walrus's --enable-ldw-opt to true for our compiles.

    The matmuls here cycle through just two stationary matrices (block 0's
    D-only and the DoubleRow L/D pair), but every self-loading InstMatmult
    re-emits an LDWEIGHTS (~105-210ns each, >100 of them). The walrus
    ldw-opt pass drops redundant consecutive loads; correctness is verified
    by the caller's rel-err check."""
    # Dead end: walrus rejects this producer's BIR with
    # "InstLdweights is not compatible with LDW optimization" when the
    # pass is enabled; redundant weight loads are instead removed by
    # _dedupe_ldweights below.
    return


def _dedupe_ldweights(nc):
    """Drop InstLdweights that reload the exact weights already resident.

    The PE stationary persists across matmuls; bass emits one InstLdweights
    per InstMatmult regardless (and walrus's own ldw-opt pass rejects this
    producer's BIR). All matmuls here cycle through just two stationaries,
    so all but the first load of each run are redundant (~167ns each, >100
    of them). Loads carrying sync are kept as same-engine NoOps so
    semaphore semantics are untouched."""
    import concourse.mybir as mybir

    for fn in nc.m.functions:
        for blk in fn.blocks:
            out = []
            last_key = None
            changed = False
            for inst in blk.instructions:
                if isinstance(inst, mybir.InstLdweights):
                    a = inst.ins[0]
                    key = (a.memref, a.offset, str(a.ap), str(a.dtype),
                           str(inst.perf_mode), inst.is_transpose)
                    if key == last_key:
                        changed = True
                        si = inst.sync_info
                        if si is not None and (si.on_wait or si.on_update):
                            out.append(mybir.InstNoOp(
                                name=f"{inst.name}-ldwskip",
                                engine=inst.engine,
                                bass_nofuse=True,
                                sync_info=si,
                            ))
                        continue
                    last_key = key
                elif isinstance(inst, mybir.InstMatmult):
                    pass  # matmults don't disturb the stationary
                elif inst.engine == getattr(mybir.EngineType, "PE",
                                            inst.engine):
                    pass
                out.append(inst)
            if changed:
                blk.instructions = out
    return nc


def _split_multi_waits(nc, maxw=1):
    """Work around a walrus limit in this image: instructions carrying more
    than ~2 sem waits die in codegen with "Too many sync wait commands".
    Move excess waits onto same-engine NoOps placed just before the
    instruction (identical sync semantics, negligible cost)."""
    import concourse.mybir as mybir

    for fn in nc.m.functions:
        for blk in fn.blocks:
            out = []
            changed = False
            for inst in blk.instructions:
                si = inst.sync_info
                if si is not None and len(si.on_wait) > maxw:
                    waits = list(si.on_wait)
                    excess, keep = waits[:-maxw], waits[-maxw:]
                    for k, w in enumerate(excess):
                        out.append(mybir.InstNoOp(
                            name=f"{inst.name}-sw{k}",
                            engine=inst.engine,
                            bass_nofuse=True,
                            sync_info=mybir.SyncInfo(on_wait=[w], on_update=[]),
                        ))
                    inst.sync_info = mybir.SyncInfo(
                        on_wait=list(keep), on_update=list(si.on_update))
                    changed = True
                out.append(inst)
            if changed:
                blk.instructions = out
    return nc


def build_program(b_loc=B_LOC, split_waits=True):
    """Per-core Bass/Tile program. Device-side x/out layout is (S, b_loc, NB).

    Measured HW facts this schedule is built around:
      - the two hardware-DGE rings (SP+ACT) sustain ~420-438 GB/s
        combined when both stream; input blocks alternate between them
        and output halves ride behind, paced with a one-block lookahead
        (pre-issuing everything measurably slows the rings);
      - consts ride the otherwise-idle gpsimd SWDGE ring so neither
        HWDGE ring has const descriptors ahead of the x stream;
      - the PE runs at 1.2GHz until it has been continuously busy for
        ~3us (then 2.4GHz), so bursts must be long and uninterrupted:
        one DoubleRow matmul per 2048-column chunk, one LDWEIGHTS each;
      - the bf16-PSUM drain TT on DVE qualifies for 2x mode, so DVE
        alone drains a block (2x 1365ns) inside the ~3us block period.

    split_waits=True post-processes for the HW compiler; pass False when the
    module is destined for CoreSim (the sim rejects the injected NoOps)."""
    import concourse.bass as bass
    import concourse.mybir as mybir
    import concourse.tile as tile

    f32 = mybir.dt.float32
    bf16 = mybir.dt.bfloat16
    fp8 = mybir.dt.float8e4
    ndrain = b_loc // BCD         # drain chunks (PSUM tiles) per block
    nmm = BCD // BC               # DoubleRow matmuls per PSUM tile
    nmm0 = BCD // BC0             # normal matmuls per PSUM tile (block 0)

    nc = bass.Bass("TRN2")
    x_h = nc.declare_dram_parameter("x", [S, b_loc * NB], fp8, False)
    # stationary pair for DoubleRow: slot 0 = L^T, slot 1 = D^T
    dl_h = nc.declare_dram_parameter("dlmat", [SB, 2, SB], fp8, False)
    # pbias pre-transposed on host -> contiguous 512B-per-partition DMA
    pb_h = nc.declare_dram_parameter("pbias", [SB, NTB, NB], bf16, False)
    out_h = nc.declare_dram_parameter("out", [S, b_loc, NB], bf16, True)

    with tile.TileContext(nc) as tc:
        with (
            tc.tile_pool(name="consts", bufs=1) as cpool,
            tc.tile_pool(name="outp", bufs=6) as opool,
            tc.tile_pool(name="tmp", bufs=6) as tpool,
            tc.tile_pool(name="psum", bufs=4, space="PSUM") as ppool,
        ):
            dl_sb = cpool.tile([SB, 2, SB], fp8, tag="dl")
            pb_sb = cpool.tile([SB, NTB, NB], bf16, tag="pb")
            # consts at the head of the then-idle scalar ring: tiny
            # (~24KB) so they cost ~0.1us of ring time
            nc.scalar.dma_start(dl_sb[:], dl_h[:])
            nc.scalar.dma_start(pb_sb[:], pb_h[:])

            # one resident tile holds all NTB sequence blocks so the
            # DoubleRow moving pair (x[blk-1], x[blk]) is a contiguous
            # [SB, 2, CW] slice
            xall = cpool.tile([SB, NTB, b_loc * NB], fp8, tag="xall")

            hb = b_loc // 2  # half-block batch split for finer DMA/sync
            DR = mybir.MatmulPerfMode.DoubleRow

            def issue_in(tb):
                # even blocks on the sync queue, odd on scalar; paced
                # one block per iteration so neither ring backs up into
                # its engine.
                eng = nc.sync if tb % 2 == 0 else nc.scalar
                r_ = slice(tb * SB, (tb + 1) * SB)
                if tb in (0, 1):
                    # first two blocks ride BOTH rings (half each) so
                    # compute starts as soon as possible
                    o_ = nc.scalar if tb % 2 == 0 else nc.sync
                    eng.dma_start(xall[:, tb, :hb * NB], x_h[r_, :hb * NB])
                    o_.dma_start(xall[:, tb, hb * NB:], x_h[r_, hb * NB:])
                elif tb == NTB - 1:
                    # split the last block so its drain starts before
                    # the full block lands
                    eng.dma_start(xall[:, tb, :hb * NB], x_h[r_, :hb * NB])
                    eng.dma_start(xall[:, tb, hb * NB:], x_h[r_, hb * NB:])
                else:
                    eng.dma_start(xall[:, tb, :], x_h[r_])

            issue_in(0)
            issue_in(1)
            for tb in range(NTB):
                for nx in (2 * tb + 2, 2 * tb + 3):
                    if nx < NTB:
                        issue_in(nx)
                r = slice(tb * SB, (tb + 1) * SB)
                ot = opool.tile([SB, b_loc, NB], bf16, tag="ot")
                bias = pb_sb[:, tb:tb + 1, :].broadcast_to((SB, BCD, NB))
                pss = {}
                for c in range(ndrain):
                    ps = ppool.tile([SB, BCD, NB], f32, tag="ps")
                    # matmuls write disjoint column slices of the tile;
                    # start=True zeroes only the 2KB PSUM BANK containing
                    # the slice, so the first matmul touching each bank
                    # must carry start (bank = 16 batches of f32)
                    if tb == 0:
                        for m in range(nmm0):
                            cs = slice((c * BCD + m * BC0) * NB,
                                       (c * BCD + (m + 1) * BC0) * NB)
                            nc.tensor.matmul(
                                ps[:, m * BC0:(m + 1) * BC0, :],
                                dl_sb[:, 1:2, :], xall[:, 0:1, cs],
                                start=True, stop=True,
                                skip_group_check=True)
                    else:
                        for m in range(nmm):
                            cs = slice((c * BCD + m * BC) * NB,
                                       (c * BCD + (m + 1) * BC) * NB)
                            nc.tensor.matmul(
                                ps[:, m * BC:(m + 1) * BC, :],
                                dl_sb[:], xall[:, tb - 1:tb + 1, cs],
                                start=(m % 2 == 0), stop=(m % 2 == 1),
                                skip_group_check=True, perf_mode=DR)
                    pss[c] = ps
                # PSUM -> SBUF bias-add (measured: DVE TT from PSUM f32
                # 1215ns/chunk, GP TT 2123ns, ACT stage-copy 1113ns).
                # GPSIMD cannot read PSUM so its chunk stages via ACT;
                # it takes c0 only (the slow path first). Giving GP a
                # second chunk per block measures consistently WORSE
                # (~54us vs ~49us): the ACT->GP chain latency outweighs
                # the DVE relief.
                gp_chunks = (0,)
                for c in range(ndrain):
                    bs = slice(c * BCD, (c + 1) * BCD)
                    if c in gp_chunks:
                        tmp = tpool.tile([SB, BCD, NB], bf16, tag="tmp")
                        nc.scalar.copy(tmp[:], pss[c][:])
                        nc.gpsimd.tensor_tensor(ot[:, bs, :], tmp[:], bias,
                                                mybir.AluOpType.add)
                    else:
                        nc.vector.tensor_tensor(ot[:, bs, :], pss[c][:], bias,
                                                mybir.AluOpType.add)
                # output halves: h0 rides scalar, h1 rides sync -- each
                # behind that queue's remaining input blocks, which are
                # all wait-free and drain first. The last block's
                # outputs go out in quarters so the ring starts as soon
                # as the first drain lands.
                if tb == NTB - 1:
                    for q in range(4):
                        qs = slice(q * BCD, (q + 1) * BCD)
                        oeng = nc.scalar if q < 2 else nc.sync
                        oeng.dma_start(out_h[r, qs, :], ot[:, qs, :])
                else:
                    nc.scalar.dma_start(out_h[r, :hb, :], ot[:, :hb, :])
                    nc.sync.dma_start(out_h[r, hb:, :], ot[:, hb:, :])
    nc = _dedupe_ldweights(nc)
    return _split_multi_waits(nc) if split_waits else nc


def to_bf16(a):
    """Convert to bfloat16 (ml_dtypes)."""
    import ml_dtypes

    return np.ascontiguousarray(np.asarray(a, dtype=F32)).astype(
        ml_dtypes.bfloat16)


def to_fp8(a):
    """Convert to TRN fp8-e4m3 (bias 7, max +-240 = ml_dtypes.float8_e4m3).

    Clip to +-240 first: values past the TRN max would round to inf."""
    import ml_dtypes

    a = np.clip(np.asarray(a, dtype=F32), -240.0, 240.0)
    return np.ascontiguousarray(a).astype(ml_dtypes.float8_e4m3)


def host_consts(alpha, beta, pos_fwd_param, pos_bwd_param, past_steps):
    """Precompute the (L^T, D^T) stationary pair and the position bias."""
    P = int(np.asarray(past_steps).reshape(-1)[0]) if np.ndim(past_steps) else int(past_steps)
    assert P <= SB, f"past_steps {P} > block size {SB} unsupported"
    a = float(np.asarray(alpha).reshape(-1)[0])
    b = float(np.asarray(beta).reshape(-1)[0])
    w = a * np.power(b, np.arange(P, dtype=np.float64))

    idx = np.arange(SB)
    km = idx[:, None] - idx[None, :]          # t - s
    D = np.where((km >= 1) & (km <= P), w[np.clip(km - 1, 0, P - 1)], 0.0)
    kml = km + SB                             # cross-block: t - s + 128
    L = np.where((kml >= 1) & (kml <= P), w[np.clip(kml - 1, 0, P - 1)], 0.0)
    # DoubleRow stationary layout [K, 2, M]: slot 0 pairs with the
    # moving slot 0 (x[blk-1]) -> L^T; slot 1 with x[blk] -> D^T
    dlpack = to_fp8(np.stack([L.T, D.T], axis=1))  # (SB, 2, SB)

    t = np.arange(S)[:, None]
    j = np.arange(NB)[None, :]
    bucket = ((t - NB * j) % S) // NB         # (S, NB)
    pf = np.asarray(pos_fwd_param, dtype=np.float64).reshape(NB)
    pbw = np.asarray(pos_bwd_param, dtype=np.float64).reshape(NB)
    pb = pf[None, :] + pbw[bucket]            # (S, NB)
    # pbias pre-transposed to (SB, NTB, NB) on the host
    pbias = to_bf16(pb.reshape(NTB, SB, NB).transpose(1, 0, 2))
    return dlpack, pbias


def reference_numpy(x, alpha, beta, pos_fwd_param, pos_bwd_param, past_steps):
    """Float64 host reference (for self-tests)."""
    P = int(past_steps)
    a = float(np.asarray(alpha).reshape(-1)[0])
    b = float(np.asarray(beta).reshape(-1)[0])
    w = a * np.power(b, np.arange(P, dtype=np.float64))
    xf = np.asarray(x, dtype=np.float64)
    Bn, Sn, Dn = xf.shape
    y = np.zeros_like(xf)
    for i in range(P):
        y[:, i + 1:, :] += w[i] * xf[:, :Sn - 1 - i, :]
    t = np.arange(Sn)[:, None]
    j = np.arange(Dn)[None, :]
    bucket = ((t - Dn * j) % Sn) // Dn
    pf = np.asarray(pos_fwd_param, dtype=np.float64).reshape(Dn)
    pbw = np.asarray(pos_bwd_param, dtype=np.float64).reshape(Dn)
    return y + pf[None, :] + pbw[bucket]


def kernel(x, alpha, beta, pos_fwd_param, pos_bwd_param, past_steps):
    _install_ntff_shim()
    _enable_ldw_opt()
    from concourse.bass_utils import run_bass_kernel_spmd

    x = np.asarray(x)
    assert x.shape == (B, S, NB), x.shape
    x = to_fp8(x)  # device datapath is fp8; halves HBM read traffic vs bf16
    dlpack, pbias = host_consts(alpha, beta, pos_fwd_param,
                                pos_bwd_param, past_steps)

    if "hw" not in _PROGRAM_CACHE:
        _PROGRAM_CACHE["hw"] = build_program(B_LOC)
    nc = _PROGRAM_CACHE["hw"]

    core_ids = list(range(NCORES))
    in_maps = [
        {
            # transposed view (S, B_LOC*NB); materialized by the runner's
            # input concat -- no extra host copy vs contiguous sharding
            "x": x[i * B_LOC:(i + 1) * B_LOC].transpose(1, 0, 2).reshape(
                S, B_LOC * NB),
            "dlmat": dlpack,
            "pbias": pbias,
        }
        for i in core_ids
    ]
    res = run_bass_kernel_spmd(nc, in_maps, core_ids)
    out = np.empty((B, S, NB), dtype=F32)
    for i in core_ids:
        out[i * B_LOC:(i + 1) * B_LOC] = (
            res.results[i]["out"].astype(F32).transpose(1, 0, 2))
    if res.exec_time_ns is not None:
        kernel.last_exec_time_ns = res.exec_time_ns
    kernel.last_results = res
    return out


kernel.last_exec_time_ns = None
kernel.last_results = None
